# revision 9
# baseline (speedup 1.0000x reference)
"""Llama decoder layer on 8 Trainium2 NeuronCores.

Strategy (v2): TP attention + single AllToAll + fully local MLP.
  - QKV/attention: tensor-parallel as baseline (2 q heads + 1 kv head per
    core, full sequence, flash-style softmax via exp + ones-matmul denom).
  - Attention outputs (still head-sharded, transposed [feat, seq]) are
    exchanged with 2 chunked AllToAlls so each core ends up with ALL heads
    for ITS OWN 512 sequence positions (2 slabs of 256).
  - o_proj, ln2, gate/up/down MLP and the final residual all run locally
    on the core's own 512 rows with full (replicated, streamed) weights.
    No ReduceScatter / AllGather / AllReduce at all.
  - Everything on the PE is bf16 (f32r measured 2x slower in accumulation
    chains); weights are pre-tiled on host into [128, *] DMA-friendly
    layouts; rmsnorm weights + 1/sqrt(hd) folded into the matmul weights.
"""
import sys
sys.path.insert(0, "/opt/trn_rl_repo")

import numpy as np

import os
import concourse.bass as bass
import concourse.mybir as mybir
import concourse.tile as tile
from concourse import bacc
from concourse.masks import make_identity

F32 = mybir.dt.float32
F32R = mybir.dt.float32r
BF16 = mybir.dt.bfloat16
AF = mybir.ActivationFunctionType
ALU = mybir.AluOpType

NCORES = 8
SEQ = 4096
HID = 2048
NH = 16
NKV = 4
HD = 128
INTER = 5632
EPS = 1e-5
THETA = 10000.0

QH = NH // NCORES            # q heads per core = 2
SCH = 512                    # seq chunk
NCH = SEQ // SCH             # 8 chunks
HB = HID // 128              # 16 hidden blocks
MB = INTER // 128            # 44 intermediate blocks
SLAB = SEQ // (2 * NCORES)   # 256: own-slab width per A2A half
OWN = 2 * SLAB               # 512 own seq positions per core


def _build():
    nc = bacc.Bacc(None, num_devices=NCORES)

    hidT_bf = nc.dram_tensor("hidT_bf", [HID, SEQ], BF16, kind="ExternalInput")
    resT = nc.dram_tensor("resT", [HID, OWN], F32, kind="ExternalInput")
    cos_t = nc.dram_tensor("cos_t", [128, SEQ], F32, kind="ExternalInput")
    sin_t = nc.dram_tensor("sin_t", [128, SEQ], F32, kind="ExternalInput")
    w_qkv_s = nc.dram_tensor("w_qkv_s", [(QH + 2) * 128, HID], BF16, kind="ExternalInput")
    wo_t = nc.dram_tensor("wo_t", [HID, HID], BF16, kind="ExternalInput")
    wgu_t = nc.dram_tensor("wgu_t", [INTER, 2 * HID], BF16, kind="ExternalInput")
    wdn_t = nc.dram_tensor("wdn_t", [HID, INTER], BF16, kind="ExternalInput")
    out = nc.dram_tensor("out", [OWN, HID], F32, kind="ExternalOutput")

    rg = [list(range(NCORES))]

    with tile.TileContext(nc) as tc:
        _emit(nc, tc, hidT_bf, resT, cos_t, sin_t,
              w_qkv_s, wo_t, wgu_t, wdn_t, out, rg)
    nc.finalize()
    return nc


def _emit(nc, tc, hidT_bf, resT, cos_t, sin_t,
          w_qkv_s, wo_t, wgu_t, wdn_t, out, rg):
    from contextlib import ExitStack
    es = ExitStack()

    # ---------------- constants ----------------
    const = es.enter_context(tc.tile_pool(name="const", bufs=1))
    ident32 = const.tile([128, 128], F32, name="ident32")
    make_identity(nc, ident32)
    identr = const.tile([128, 128], F32R, name="identr")
    nc.vector.tensor_copy(identr[:], ident32[:])
    identb = const.tile([128, 128], BF16, name="identb")
    nc.vector.tensor_copy(identb[:], ident32[:])
    ones32 = const.tile([128, 128], F32, name="ones32")
    nc.vector.memset(ones32[:], 1.0)
    # [128,128] all-ones stationary: column-sum matmuls produce the result
    # replicated on every partition (wide reciprocal, no partition_broadcast)
    ones_b = const.tile([128, 128], BF16, name="ones_b")
    nc.vector.tensor_copy(ones_b[:], ones32[:])
    epsc = const.tile([128, 1], F32, name="epsc")
    nc.vector.memset(epsc[:], EPS)
    # causal masks for the 4 diagonal t-blocks of a 512-wide q chunk
    masks = []
    for j in range(4):
        m32 = const.tile([128, SCH], F32, name="m32scratch", tag="m32scratch")
        nc.vector.memset(m32[:], 1.0)
        nc.gpsimd.affine_select(
            out=m32[:], in_=m32[:], compare_op=ALU.is_ge,
            fill=0.0, base=-j * 128, channel_multiplier=-1, pattern=[[1, SCH]],
        )
        mj = const.tile([128, SCH], BF16, name=f"mask_{j}")
        nc.vector.tensor_copy(mj[:], m32[:])
        masks.append(mj)

    wk = es.enter_context(tc.tile_pool(name="wk", bufs=4))

    # DRAM scratch for the two AllToAlls
    dr_a2a = es.enter_context(tc.tile_pool(name="dr_a2a", bufs=1, space="DRAM"))
    a2a_in = [dr_a2a.tile([HID, SLAB], BF16, name=f"a2a_in{i}", tag=f"a2a_in{i}")
              for i in range(2)]
    a2a_out = [dr_a2a.tile([HID, SLAB], BF16, name=f"a2a_out{i}", tag=f"a2a_out{i}")
               for i in range(2)]

    # ============ Phase A+B interleaved: QKV+RoPE then attention, per chunk ====
    ab = ExitStack()
    ps_mm = ab.enter_context(tc.tile_pool(name="ps_mm", bufs=2, space="PSUM"))
    ps_s = ab.enter_context(tc.tile_pool(name="ps_s", bufs=2, space="PSUM"))
    ps_tr = ab.enter_context(tc.tile_pool(name="ps_tr", bufs=1, space="PSUM"))
    ps_o = ab.enter_context(tc.tile_pool(name="ps_o", bufs=2, space="PSUM"))
    ps_sd = ab.enter_context(tc.tile_pool(name="ps_sd", bufs=1, space="PSUM"))

    att = ab.enter_context(tc.tile_pool(name="att", bufs=1))
    wqp = ab.enter_context(tc.tile_pool(name="wq", bufs=1))
    trig = ab.enter_context(tc.tile_pool(name="trig", bufs=2))
    xp = ab.enter_context(tc.tile_pool(name="xp", bufs=2))
    aw = ab.enter_context(tc.tile_pool(name="aw", bufs=2))
    rw = ab.enter_context(tc.tile_pool(name="rw", bufs=2))
    ew = ab.enter_context(tc.tile_pool(name="ew", bufs=3))
    atw = ab.enter_context(tc.tile_pool(name="atw", bufs=2))

    wq_sb = wqp.tile([128, (QH + 2) * HID], BF16, name="wq_sb")
    for cb in range(QH + 2):
        nc.sync.dma_start(wq_sb[:, cb * HID:(cb + 1) * HID],
                          w_qkv_s[cb * 128:(cb + 1) * 128, :])

    qT_c = [[att.tile([128, SCH], BF16, name=f"qT{h}_{c}") for c in range(NCH)]
            for h in range(QH)]
    kT_c = [att.tile([128, SCH], BF16, name=f"kT_{c}") for c in range(NCH)]
    vN_c = [att.tile([128, SCH], BF16, name=f"vN_{c}") for c in range(NCH)]

    # ---- Phase A: ln1 + QKV + RoPE for all chunks (dense PE chains) ----
    for c in range(NCH):
        scol = slice(c * SCH, (c + 1) * SCH)
        cos_sb = trig.tile([128, SCH], F32, name="cos_sb", tag="cos_sb")
        sin_sb = trig.tile([128, SCH], F32, name="sin_sb", tag="sin_sb")
        nc.sync.dma_start(cos_sb[:], cos_t[:, scol])
        nc.sync.dma_start(sin_sb[:], sin_t[:, scol])
        xts = []
        for hb in range(HB):
            xt = xp.tile([128, SCH], BF16, name="xt", tag=f"xt{hb}")
            nc.sync.dma_start(xt[:], hidT_bf[hb * 128:(hb + 1) * 128, scol])
            xts.append(xt)

        # sum of squares over hidden dim; ones128 stationary -> result on
        # every partition (sq on Pool engine to keep DVE free for rope)
        ss_ps = ps_sd.tile([128, SCH], F32, name="ss_ps", tag="sd")
        for hb in range(HB):
            sq = aw.tile([128, SCH], BF16, name="sq", tag="sq")
            nc.gpsimd.tensor_mul(sq[:], xts[hb][:], xts[hb][:])
            nc.tensor.matmul(ss_ps[:], ones_b[:], sq[:],
                             start=(hb == 0), stop=(hb == HB - 1),
                             skip_group_check=True)
        rinv_bc = aw.tile([128, SCH], F32, name="rinv_bc", tag="rinv_bc")
        nc.scalar.activation(rinv_bc[:], ss_ps[:], AF.Rsqrt, scale=1.0 / HID,
                             bias=epsc[:, :])

        # qkv matmuls (bf16, 16-chain per output block)
        for cb in range(QH + 2):
            qkv_ps = ps_mm.tile([128, SCH], F32, name="qkv_ps", tag="mm")
            for hb in range(HB):
                nc.tensor.matmul(qkv_ps[:],
                                 wq_sb[:, cb * HID + hb * 128:cb * HID + (hb + 1) * 128],
                                 xts[hb][:],
                                 start=(hb == 0), stop=(hb == HB - 1),
                                 skip_group_check=True)
            if cb < QH + 1:
                # neox rope: dst = raw*cos + swap(raw)*sin (sin rows 0-63 negated)
                raw = rw.tile([128, SCH], F32, name="raw", tag="raw")
                nc.vector.tensor_mul(raw[:], qkv_ps[:], rinv_bc[:])
                swp = rw.tile([128, SCH], F32, name="swp", tag="swp")
                nc.sync.dma_start(swp[0:64, :], raw[64:128, :])
                nc.sync.dma_start(swp[64:128, :], raw[0:64, :])
                t1 = rw.tile([128, SCH], F32, name="t1", tag="t1")
                t2 = rw.tile([128, SCH], F32, name="t2", tag="t2")
                nc.vector.tensor_mul(t1[:], raw[:], cos_sb[:])
                nc.vector.tensor_mul(t2[:], swp[:], sin_sb[:])
                dst = qT_c[cb][c] if cb < QH else kT_c[c]
                nc.vector.tensor_add(dst[:], t1[:], t2[:])
            else:
                # V: rescale, then transpose [d, s] -> natural [t, d] blocks
                vb = rw.tile([128, SCH], BF16, name="vb", tag="vb")
                nc.vector.tensor_mul(vb[:], qkv_ps[:], rinv_bc[:])
                for i in range(SCH // 128):
                    tp = ps_tr.tile([128, 128], BF16, name="tp", tag="tp")
                    nc.tensor.transpose(tp[:], vb[:, i * 128:(i + 1) * 128], identb[:])
                    nc.vector.tensor_copy(vN_c[c][:, i * 128:(i + 1) * 128], tp[:])

    # ---- Phase B: attention per q-chunk ----
    for c in range(NCH):
        ntb = 4 * c + 4
        for h in range(QH):
            o_ps = ps_o.tile([128, SCH], F32, name="o_ps", tag="o")
            den_ps = ps_sd.tile([128, SCH], F32, name="den_ps", tag="sd")
            eTs = [None, None]  # 1-deep software pipeline: scores ahead of PV
            for tb in range(ntb + 1):
                if tb < ntb:
                    s_ps = ps_s.tile([128, SCH], F32, name="s_ps", tag="s")
                    nc.tensor.matmul(s_ps[:],
                                     kT_c[tb // 4][:, (tb % 4) * 128:(tb % 4 + 1) * 128],
                                     qT_c[h][c][:], start=True, stop=True,
                                     skip_group_check=True)
                    eT = ew.tile([128, SCH], BF16, name="eT", tag="eT")
                    nc.scalar.activation(eT[:], s_ps[:], AF.Exp)
                    j = tb - 4 * c
                    if j >= 0:
                        eTm = ew.tile([128, SCH], BF16, name="eTm", tag="eTm")
                        nc.vector.tensor_mul(eTm[:], eT[:], masks[j][:])
                        eT = eTm
                    eTs[tb % 2] = eT
                if tb > 0:
                    pb = tb - 1
                    eT = eTs[pb % 2]
                    nc.tensor.matmul(o_ps[:],
                                     vN_c[pb // 4][:, (pb % 4) * 128:(pb % 4 + 1) * 128],
                                     eT[:], start=(pb == 0), stop=(pb == ntb - 1),
                                     skip_group_check=True)
                    nc.tensor.matmul(den_ps[:], ones_b[:], eT[:],
                                     start=(pb == 0), stop=(pb == ntb - 1),
                                     skip_group_check=True)
            dinv = atw.tile([128, SCH], F32, name="dinv", tag="dinv")
            nc.vector.reciprocal(dinv[:], den_ps[:])
            aT = atw.tile([128, SCH], BF16, name="aT", tag="aT")
            nc.vector.tensor_mul(aT[:], o_ps[:], dinv[:])
            # scatter into a2a_in: slab i = c//4, shards j0/j0+1 within slab
            i = c // 4
            j0 = 2 * (c % 4)
            nc.sync.dma_start(
                a2a_in[i][(256 * j0 + 128 * h):(256 * j0 + 128 * (h + 1)), :],
                aT[:, 0:SLAB])
            nc.sync.dma_start(
                a2a_in[i][(256 * (j0 + 1) + 128 * h):(256 * (j0 + 1) + 128 * (h + 1)), :],
                aT[:, SLAB:2 * SLAB])

        if c == 3 or c == 7:
            i = c // 4
            nc.gpsimd.collective_compute(
                "AllToAll", ALU.bypass, replica_groups=rg,
                ins=[a2a_in[i][:].opt()], outs=[a2a_out[i][:].opt()])

    ab.close()

    # ============ Phase C: o_proj + residual + ln2 (own 512 cols) ============
    cd = ExitStack()
    ps_op = cd.enter_context(tc.tile_pool(name="ps_op", bufs=2, space="PSUM"))
    ps_s2 = cd.enter_context(tc.tile_pool(name="ps_s2", bufs=1, space="PSUM"))
    ps_g = cd.enter_context(tc.tile_pool(name="ps_g", bufs=1, space="PSUM"))
    ps_u = cd.enter_context(tc.tile_pool(name="ps_u", bufs=1, space="PSUM"))
    ps_dn = cd.enter_context(tc.tile_pool(name="ps_dn", bufs=2, space="PSUM"))
    ps_t2 = cd.enter_context(tc.tile_pool(name="ps_t2", bufs=1, space="PSUM"))

    h2p = cd.enter_context(tc.tile_pool(name="h2p", bufs=1))
    xnp = cd.enter_context(tc.tile_pool(name="xnp", bufs=1))
    dw = cd.enter_context(tc.tile_pool(name="dw", bufs=2))

    h2s = []
    xns = []
    with tc.tile_pool(name="arp", bufs=1) as arp, \
         tc.tile_pool(name="rp", bufs=1) as rp, \
         tc.tile_pool(name="wop", bufs=2) as wop:
        rTs = []
        for hb in range(HB):
            rT = rp.tile([128, OWN], F32, name="rT", tag=f"rT{hb}")
            nc.sync.dma_start(rT[:], resT[hb * 128:(hb + 1) * 128, :])
            rTs.append(rT)
        attn_rhs = []
        for fb in range(HB):
            ar = arp.tile([128, OWN], BF16, name="ar", tag=f"ar{fb}")
            nc.sync.dma_start(ar[:, 0:SLAB], a2a_out[0][fb * 128:(fb + 1) * 128, :])
            nc.sync.dma_start(ar[:, SLAB:OWN], a2a_out[1][fb * 128:(fb + 1) * 128, :])
            attn_rhs.append(ar)

        ss2_ps = ps_s2.tile([128, OWN], F32, name="ss2_ps", tag="s2")
        for n in range(HB):
            won = wop.tile([128, HID], BF16, name="won", tag="won")
            nc.sync.dma_start(won[:], wo_t[n * 128:(n + 1) * 128, :])
            o_acc = ps_op.tile([128, OWN], F32, name="o_acc", tag="op")
            for k in range(HB):
                nc.tensor.matmul(o_acc[:], won[:, k * 128:(k + 1) * 128],
                                 attn_rhs[k][:],
                                 start=(k == 0), stop=(k == HB - 1),
                                 skip_group_check=True)
            h2 = h2p.tile([128, OWN], F32, name="h2", tag=f"h2{n}")
            nc.vector.tensor_add(h2[:], o_acc[:], rTs[n][:])
            sq2 = dw.tile([128, OWN], BF16, name="sq2", tag="sq2")
            nc.gpsimd.tensor_mul(sq2[:], h2[:], h2[:])
            nc.tensor.matmul(ss2_ps[:], ones_b[:], sq2[:],
                             start=(n == 0), stop=(n == HB - 1),
                             skip_group_check=True)
            h2s.append(h2)

        rinv2_bc = dw.tile([128, OWN], F32, name="rinv2_bc", tag="r2bc")
        nc.scalar.activation(rinv2_bc[:], ss2_ps[:], AF.Rsqrt, scale=1.0 / HID,
                             bias=epsc[:, :])
        for n in range(HB):
            xn = xnp.tile([128, OWN], BF16, name="xn", tag=f"xn{n}")
            nc.vector.tensor_mul(xn[:], h2s[n][:], rinv2_bc[:])
            xns.append(xn)

    # ============ Phase D: gate/up + silu*up + down + residual + out ========
    with tc.tile_pool(name="wgup", bufs=3) as wgup, \
         tc.tile_pool(name="wdnp", bufs=2) as wdnp, \
         tc.tile_pool(name="hTp", bufs=1) as hTp:
        hTs = []
        for m in range(MB):
            wgum = wgup.tile([128, 2 * HID], BF16, name="wgum", tag="wgum")
            nc.sync.dma_start(wgum[:], wgu_t[m * 128:(m + 1) * 128, :])
            g_ps = ps_g.tile([128, OWN], F32, name="g_ps", tag="g")
            for hb in range(HB):
                nc.tensor.matmul(g_ps[:], wgum[:, hb * 128:(hb + 1) * 128],
                                 xns[hb][:],
                                 start=(hb == 0), stop=(hb == HB - 1),
                                 skip_group_check=True)
            u_ps = ps_u.tile([128, OWN], F32, name="u_ps", tag="u")
            for hb in range(HB):
                nc.tensor.matmul(u_ps[:], wgum[:, HID + hb * 128:HID + (hb + 1) * 128],
                                 xns[hb][:],
                                 start=(hb == 0), stop=(hb == HB - 1),
                                 skip_group_check=True)
            sg = dw.tile([128, OWN], F32, name="sg", tag="sg")
            nc.scalar.activation(sg[:], g_ps[:], AF.Silu)
            hT = hTp.tile([128, OWN], BF16, name="hT", tag=f"hT{m}")
            nc.vector.tensor_mul(hT[:], sg[:], u_ps[:])
            hTs.append(hT)

        for n in range(HB):
            wdnn = wdnp.tile([128, INTER], BF16, name="wdnn", tag="wdnn")
            nc.sync.dma_start(wdnn[:], wdn_t[n * 128:(n + 1) * 128, :])
            dn_ps = ps_dn.tile([128, OWN], F32, name="dn_ps", tag="dn")
            for k in range(MB):
                nc.tensor.matmul(dn_ps[:], wdnn[:, k * 128:(k + 1) * 128],
                                 hTs[k][:],
                                 start=(k == 0), stop=(k == MB - 1),
                                 skip_group_check=True)
            y = dw.tile([128, OWN], F32R, name="y", tag="y")
            nc.vector.tensor_add(y[:], dn_ps[:], h2s[n][:])
            for qb in range(OWN // 128):
                tp = ps_t2.tile([128, 128], F32R, name="tp2", tag="t2")
                nc.tensor.transpose(tp[:], y[:, qb * 128:(qb + 1) * 128], identr[:])
                oc = dw.tile([128, 128], F32, name="oc", tag="oc")
                nc.vector.tensor_copy(oc[:], tp.bitcast(F32)[:])
                nc.sync.dma_start(
                    out[qb * 128:(qb + 1) * 128, n * 128:(n + 1) * 128], oc[:])

    cd.close()
    es.close()


# ---------------- host side ----------------

_CACHE = {}


def _get_runner():
    if "runner" in _CACHE:
        return _CACHE["runner"]
    import jax
    from jax.sharding import Mesh, PartitionSpec
    from jax.experimental.shard_map import shard_map
    from concourse import bass2jax

    nc = _build()
    bass2jax.install_neuronx_cc_hook()

    in_names = []
    out_names = []
    out_avals = []
    zero_shapes = []
    for alloc in nc.m.functions[0].allocations:
        if not isinstance(alloc, mybir.MemoryLocationSet):
            continue
        name = alloc.memorylocations[0].name
        if alloc.kind == "ExternalInput":
            if nc.partition_id_tensor is None or name != nc.partition_id_tensor.name:
                in_names.append(name)
        elif alloc.kind == "ExternalOutput":
            out_names.append(name)
            shape = tuple(alloc.tensor_shape)
            dtype = mybir.dt.np(alloc.dtype)
            out_avals.append(jax.core.ShapedArray(shape, dtype))
            zero_shapes.append((shape, dtype))
    n_params = len(in_names)
    full_in_names = list(in_names) + list(out_names)
    if nc.partition_id_tensor is not None:
        full_in_names.append(nc.partition_id_tensor.name)
    donate = tuple(range(n_params, n_params + len(out_names)))

    def _body(*args):
        operands = list(args)
        if nc.partition_id_tensor is not None:
            operands.append(bass2jax.partition_id_tensor())
        outs = bass2jax._bass_exec_p.bind(
            *operands,
            out_avals=tuple(out_avals),
            in_names=tuple(full_in_names),
            out_names=tuple(out_names),
            lowering_input_output_aliases=(),
            sim_require_finite=True,
            sim_require_nnan=True,
            nc=nc,
        )
        return tuple(outs)

    devices = jax.devices()[:NCORES]
    mesh = Mesh(np.asarray(devices), ("core",))
    in_specs = (PartitionSpec("core"),) * (n_params + len(out_names))
    out_specs = (PartitionSpec("core"),) * len(out_names)
    sharded = jax.jit(
        shard_map(_body, mesh=mesh, in_specs=in_specs, out_specs=out_specs,
                  check_rep=False),
        donate_argnums=donate, keep_unused=True,
    )
    runner = dict(fn=sharded, in_names=in_names, out_names=out_names,
                  zero_shapes=zero_shapes, out_avals=out_avals)
    _CACHE["runner"] = runner
    return runner


def _prep_inputs(positions, hidden_states, ln1_w, ln2_w, w_qkv, w_o, w_gate_up, w_down):
    """Build per-core input dicts (list of NCORES dicts, numpy)."""
    import ml_dtypes
    BF = ml_dtypes.bfloat16
    hs = np.asarray(hidden_states, dtype=np.float32)
    pos = np.asarray(positions, dtype=np.float64)
    ln1 = np.asarray(ln1_w, dtype=np.float32)
    ln2 = np.asarray(ln2_w, dtype=np.float32)
    wq = np.asarray(w_qkv, dtype=np.float32)
    wo = np.asarray(w_o, dtype=np.float32)
    wgu = np.asarray(w_gate_up, dtype=np.float32)
    wdn = np.asarray(w_down, dtype=np.float32)

    hidT = np.ascontiguousarray(hs.T)                      # [HID, SEQ] f32
    hidT_bf = hidT.astype(BF)
    inv_freq = 1.0 / (THETA ** (np.arange(0, HD, 2, dtype=np.float64) / HD))
    freqs = pos[:, None] * inv_freq[None, :]               # [SEQ, 64]
    cos_h = np.cos(freqs).T.astype(np.float32)             # [64, SEQ]
    sin_h = np.sin(freqs).T.astype(np.float32)
    cos_t = np.ascontiguousarray(np.concatenate([cos_h, cos_h], axis=0))
    sin_t = np.ascontiguousarray(np.concatenate([-sin_h, sin_h], axis=0))

    wq_eff = wq * ln1[:, None]
    wgu_eff = wgu * ln2[:, None]
    scale = HD ** -0.5
    q_size = NH * HD
    kv_size = NKV * HD

    # shared pre-tiled weights (lhsT block layouts)
    wo_t = np.ascontiguousarray(
        wo.reshape(HB, 128, HB, 128).transpose(2, 1, 0, 3).reshape(HID, HID)
    ).astype(BF)
    G = wgu_eff[:, :INTER].reshape(HB, 128, MB, 128).transpose(2, 1, 0, 3)
    U = wgu_eff[:, INTER:].reshape(HB, 128, MB, 128).transpose(2, 1, 0, 3)
    wgu_t = np.ascontiguousarray(
        np.stack([G, U], axis=2).reshape(INTER, 2 * HID)).astype(BF)
    wdn_t = np.ascontiguousarray(
        wdn.reshape(MB, 128, HB, 128).transpose(2, 1, 0, 3).reshape(HID, INTER)
    ).astype(BF)

    per_core = []
    for c in range(NCORES):
        kvh = c // 2
        q_cols = wq_eff[:, QH * c * HD:QH * (c + 1) * HD] * scale
        k_cols = wq_eff[:, q_size + kvh * HD:q_size + (kvh + 1) * HD]
        v_cols = wq_eff[:, q_size + kv_size + kvh * HD:q_size + kv_size + (kvh + 1) * HD]
        W = np.concatenate([q_cols, k_cols, v_cols], axis=1)   # [HID, 512]
        w_qkv_s = np.ascontiguousarray(
            W.reshape(HB, 128, QH + 2, 128).transpose(2, 1, 0, 3)
            .reshape((QH + 2) * 128, HID)).astype(BF)
        own = np.concatenate(
            [hidT[:, SLAB * c:SLAB * (c + 1)],
             hidT[:, SEQ // 2 + SLAB * c:SEQ // 2 + SLAB * (c + 1)]], axis=1)
        per_core.append({
            "hidT_bf": hidT_bf, "resT": np.ascontiguousarray(own),
            "cos_t": cos_t, "sin_t": sin_t,
            "w_qkv_s": w_qkv_s, "wo_t": wo_t, "wgu_t": wgu_t, "wdn_t": wdn_t,
        })
    return per_core


def kernel(positions, hidden_states, ln1_w, ln2_w, w_qkv, w_o, w_gate_up, w_down):
    runner = _get_runner()
    per_core = _prep_inputs(positions, hidden_states, ln1_w, ln2_w,
                            w_qkv, w_o, w_gate_up, w_down)
    concat_in = [
        np.concatenate([np.asarray(per_core[c][name]) for c in range(NCORES)], axis=0)
        for name in runner["in_names"]
    ]
    concat_zeros = [
        np.zeros((NCORES * s[0],) + tuple(s[1:]), d)
        for (s, d) in runner["zero_shapes"]
    ]
    outs = runner["fn"](*concat_in, *concat_zeros)
    out = np.asarray(outs[0]).reshape(NCORES, OWN, HID)
    full = np.empty((SEQ, HID), dtype=np.float32)
    for c in range(NCORES):
        full[SLAB * c:SLAB * (c + 1)] = out[c][:SLAB]
        full[SEQ // 2 + SLAB * c:SEQ // 2 + SLAB * (c + 1)] = out[c][SLAB:]
    return full


if __name__ == "__main__":
    print("building...")
    _get_runner()
    print("built ok")


# revision 19
# speedup vs baseline: 1.2050x; 1.2050x over previous
"""Llama decoder layer on 8 Trainium2 NeuronCores.

Strategy (v2): TP attention + single AllToAll + fully local MLP.
  - QKV/attention: tensor-parallel as baseline (2 q heads + 1 kv head per
    core, full sequence, flash-style softmax via exp + ones-matmul denom).
  - Attention outputs (still head-sharded, transposed [feat, seq]) are
    exchanged with 2 chunked AllToAlls so each core ends up with ALL heads
    for ITS OWN 512 sequence positions (2 slabs of 256).
  - o_proj, ln2, gate/up/down MLP and the final residual all run locally
    on the core's own 512 rows with full (replicated, streamed) weights.
    No ReduceScatter / AllGather / AllReduce at all.
  - Everything on the PE is bf16 (f32r measured 2x slower in accumulation
    chains); weights are pre-tiled on host into [128, *] DMA-friendly
    layouts; rmsnorm weights + 1/sqrt(hd) folded into the matmul weights.
"""
import sys
sys.path.insert(0, "/opt/trn_rl_repo")

import numpy as np

import os
import concourse.bass as bass
import concourse.mybir as mybir
import concourse.tile as tile
from concourse import bacc
from concourse.masks import make_identity

F32 = mybir.dt.float32
F32R = mybir.dt.float32r
BF16 = mybir.dt.bfloat16
AF = mybir.ActivationFunctionType
ALU = mybir.AluOpType

NCORES = 8
SEQ = 4096
HID = 2048
NH = 16
NKV = 4
HD = 128
INTER = 5632
EPS = 1e-5
THETA = 10000.0

QH = NH // NCORES            # q heads per core = 2
SCH = 512                    # seq chunk
NCH = SEQ // SCH             # 8 chunks
HB = HID // 128              # 16 hidden blocks
MB = INTER // 128            # 44 intermediate blocks
NSLAB = 4                    # number of chunked AllToAlls (1024-col q slabs)
SLAB = SEQ // (NSLAB * NCORES)  # 128: own-slab width per A2A
OWN = NSLAB * SLAB           # 512 own seq positions per core


def _build():
    nc = bacc.Bacc(None, num_devices=NCORES)

    hidT_bf = nc.dram_tensor("hidT_bf", [HID, SEQ], BF16, kind="ExternalInput")
    resT = nc.dram_tensor("resT", [HID, OWN], F32, kind="ExternalInput")
    cos_t = nc.dram_tensor("cos_t", [128, SEQ], F32, kind="ExternalInput")
    sin_t = nc.dram_tensor("sin_t", [128, SEQ], F32, kind="ExternalInput")
    w_qkv_s = nc.dram_tensor("w_qkv_s", [(QH + 2) * 128, HID], BF16, kind="ExternalInput")
    wo_t = nc.dram_tensor("wo_t", [HID, HID], BF16, kind="ExternalInput")
    wgu_t = nc.dram_tensor("wgu_t", [INTER, 2 * HID], BF16, kind="ExternalInput")
    wdn_t = nc.dram_tensor("wdn_t", [HID, INTER], BF16, kind="ExternalInput")
    out = nc.dram_tensor("out", [OWN, HID], F32, kind="ExternalOutput")

    rg = [list(range(NCORES))]

    with tile.TileContext(nc) as tc:
        _emit(nc, tc, hidT_bf, resT, cos_t, sin_t,
              w_qkv_s, wo_t, wgu_t, wdn_t, out, rg)
    nc.finalize()
    return nc


def _emit(nc, tc, hidT_bf, resT, cos_t, sin_t,
          w_qkv_s, wo_t, wgu_t, wdn_t, out, rg):
    from contextlib import ExitStack
    es = ExitStack()

    # ---------------- constants ----------------
    const = es.enter_context(tc.tile_pool(name="const", bufs=1))
    ident32 = const.tile([128, 128], F32, name="ident32")
    make_identity(nc, ident32)
    identr = const.tile([128, 128], F32R, name="identr")
    nc.vector.tensor_copy(identr[:], ident32[:])
    identb = const.tile([128, 128], BF16, name="identb")
    nc.vector.tensor_copy(identb[:], ident32[:])
    ones32 = const.tile([128, 128], F32, name="ones32")
    nc.vector.memset(ones32[:], 1.0)
    # [128,128] all-ones stationary: column-sum matmuls produce the result
    # replicated on every partition (wide reciprocal, no partition_broadcast)
    ones_b = const.tile([128, 128], BF16, name="ones_b")
    nc.vector.tensor_copy(ones_b[:], ones32[:])
    epsc = const.tile([128, 1], F32, name="epsc")
    nc.vector.memset(epsc[:], EPS)
    # causal masks for the diagonal t-block PAIRS of a 512-wide q chunk:
    # masks2[jj] is [128, 1024] bf16 covering t-blocks 2jj | 2jj+1
    masks2 = []
    for jj in range(2):
        m32 = const.tile([128, 2 * SCH], F32, name="m32scratch", tag="m32scratch")
        nc.vector.memset(m32[:], 1.0)
        for half in range(2):
            j = 2 * jj + half
            nc.gpsimd.affine_select(
                out=m32[:, half * SCH:(half + 1) * SCH],
                in_=m32[:, half * SCH:(half + 1) * SCH], compare_op=ALU.is_ge,
                fill=0.0, base=-j * 128, channel_multiplier=-1, pattern=[[1, SCH]],
            )
        mj = const.tile([128, 2 * SCH], BF16, name=f"mask2_{jj}")
        nc.vector.tensor_copy(mj[:], m32[:])
        masks2.append(mj)

    # DRAM scratch for the four chunked AllToAlls (one per 1024-col q slab)
    dr_a2a = es.enter_context(tc.tile_pool(name="dr_a2a", bufs=1, space="DRAM"))
    a2a_in = [dr_a2a.tile([HID, SLAB], BF16, name=f"a2a_in{i}", tag=f"a2a_in{i}")
              for i in range(NSLAB)]
    a2a_out = [dr_a2a.tile([HID, SLAB], BF16, name=f"a2a_out{i}", tag=f"a2a_out{i}")
               for i in range(NSLAB)]

    # ============ Phase A+B: QKV+RoPE for all chunks, then attention ====
    ab = ExitStack()
    pa = ExitStack()
    ps_mm = pa.enter_context(tc.tile_pool(name="ps_mm", bufs=2, space="PSUM"))
    ps_tr = pa.enter_context(tc.tile_pool(name="ps_tr", bufs=1, space="PSUM"))
    ps_ss = pa.enter_context(tc.tile_pool(name="ps_ss", bufs=1, space="PSUM"))

    att = ab.enter_context(tc.tile_pool(name="att", bufs=1))
    wqp = ab.enter_context(tc.tile_pool(name="wq", bufs=1))
    trig = ab.enter_context(tc.tile_pool(name="trig", bufs=2))
    xp = ab.enter_context(tc.tile_pool(name="xp", bufs=2))
    aw = ab.enter_context(tc.tile_pool(name="aw", bufs=2))
    rw = ab.enter_context(tc.tile_pool(name="rw", bufs=2))
    ew = ab.enter_context(tc.tile_pool(name="ew", bufs=3))
    atw = ab.enter_context(tc.tile_pool(name="atw", bufs=2))

    wq_sb = wqp.tile([128, (QH + 2) * HID], BF16, name="wq_sb")
    for cb in range(QH + 2):
        nc.sync.dma_start(wq_sb[:, cb * HID:(cb + 1) * HID],
                          w_qkv_s[cb * 128:(cb + 1) * 128, :])

    qT_c = [[att.tile([128, SCH], BF16, name=f"qT{h}_{c}") for c in range(NCH)]
            for h in range(QH)]
    kT_c = [att.tile([128, SCH], BF16, name=f"kT_{c}") for c in range(NCH)]
    vN_c = [att.tile([128, SCH], BF16, name=f"vN_{c}") for c in range(NCH)]

    # ---- Phase A: ln1 + QKV + RoPE for all chunks (dense PE chains) ----
    for c in range(NCH):
        scol = slice(c * SCH, (c + 1) * SCH)
        cos_sb = trig.tile([128, SCH], F32, name="cos_sb", tag="cos_sb")
        sin_sb = trig.tile([128, SCH], F32, name="sin_sb", tag="sin_sb")
        nc.sync.dma_start(cos_sb[:], cos_t[:, scol])
        nc.sync.dma_start(sin_sb[:], sin_t[:, scol])
        xts = []
        for hb in range(HB):
            xt = xp.tile([128, SCH], BF16, name="xt", tag=f"xt{hb}")
            nc.sync.dma_start(xt[:], hidT_bf[hb * 128:(hb + 1) * 128, scol])
            xts.append(xt)

        # sum of squares over hidden dim; ones128 stationary -> result on
        # every partition (wide reciprocal, no partition broadcast)
        ss_ps = ps_ss.tile([128, SCH], F32, name="ss_ps", tag="ss")
        for hb in range(HB):
            sq = aw.tile([128, SCH], BF16, name="sq", tag="sq")
            nc.vector.tensor_mul(sq[:], xts[hb][:], xts[hb][:])
            nc.tensor.matmul(ss_ps[:], ones_b[:], sq[:],
                             start=(hb == 0), stop=(hb == HB - 1),
                             skip_group_check=True)
        stdv = aw.tile([128, SCH], F32, name="stdv", tag="stdv")
        nc.scalar.activation(stdv[:], ss_ps[:], AF.Sqrt, scale=1.0 / HID,
                             bias=epsc[:, :])
        rinv_bc = aw.tile([128, SCH], F32, name="rinv_bc", tag="rinv_bc")
        nc.vector.reciprocal(rinv_bc[:], stdv[:])

        # qkv matmuls (bf16, 16-chain per output block)
        for cb in range(QH + 2):
            qkv_ps = ps_mm.tile([128, SCH], F32, name="qkv_ps", tag="mm")
            for hb in range(HB):
                nc.tensor.matmul(qkv_ps[:],
                                 wq_sb[:, cb * HID + hb * 128:cb * HID + (hb + 1) * 128],
                                 xts[hb][:],
                                 start=(hb == 0), stop=(hb == HB - 1),
                                 skip_group_check=True)
            if cb < QH + 1:
                # neox rope: dst = raw*cos + swap(raw)*sin (sin rows 0-63 negated)
                raw = rw.tile([128, SCH], F32, name="raw", tag="raw")
                nc.vector.tensor_mul(raw[:], qkv_ps[:], rinv_bc[:])
                swp = rw.tile([128, SCH], F32, name="swp", tag="swp")
                nc.sync.dma_start(swp[0:64, :], raw[64:128, :])
                nc.sync.dma_start(swp[64:128, :], raw[0:64, :])
                t1 = rw.tile([128, SCH], F32, name="t1", tag="t1")
                t2 = rw.tile([128, SCH], F32, name="t2", tag="t2")
                nc.vector.tensor_mul(t1[:], raw[:], cos_sb[:])
                nc.vector.tensor_mul(t2[:], swp[:], sin_sb[:])
                dst = qT_c[cb][c] if cb < QH else kT_c[c]
                nc.vector.tensor_add(dst[:], t1[:], t2[:])
            else:
                # V: rescale, then transpose [d, s] -> natural [t, d] blocks
                vb = rw.tile([128, SCH], BF16, name="vb", tag="vb")
                nc.vector.tensor_mul(vb[:], qkv_ps[:], rinv_bc[:])
                for i in range(SCH // 128):
                    tp = ps_tr.tile([128, 128], BF16, name="tp", tag="tp")
                    nc.tensor.transpose(tp[:], vb[:, i * 128:(i + 1) * 128], identb[:])
                    nc.vector.tensor_copy(vN_c[c][:, i * 128:(i + 1) * 128], tp[:])

    pa.close()

    # ---- Phase B: attention per q-chunk (paired t-blocks, fused exp) ----
    pb_st = ExitStack()
    ps_s = pb_st.enter_context(tc.tile_pool(name="ps_s", bufs=2, space="PSUM"))
    ps_o = pb_st.enter_context(tc.tile_pool(name="ps_o", bufs=2, space="PSUM"))
    ps_den = pb_st.enter_context(tc.tile_pool(name="ps_den", bufs=1, space="PSUM"))

    for c in range(NCH):
        ntb = 4 * c + 4
        npair = ntb // 2
        for h in range(QH):
            o_ps = ps_o.tile([128, SCH], F32, name="o_ps", tag="o")
            den_ps = ps_den.tile([128, SCH], F32, name="den_ps", tag="den")
            e2s = [None, None]  # 1-deep software pipeline: scores ahead of PV
            for p in range(npair + 1):
                if p < npair:
                    s2 = ps_s.tile([128, 2 * SCH], F32, name="s2", tag="s")
                    for half in range(2):
                        tb = 2 * p + half
                        nc.tensor.matmul(
                            s2[:, half * SCH:(half + 1) * SCH],
                            kT_c[tb // 4][:, (tb % 4) * 128:(tb % 4 + 1) * 128],
                            qT_c[h][c][:], start=True, stop=True,
                            skip_group_check=True)
                    e2 = ew.tile([128, 2 * SCH], BF16, name="e2", tag="e2")
                    nc.scalar.activation(e2[:], s2[:], AF.Exp)
                    jj = p - 2 * c
                    if jj >= 0:
                        e2m = ew.tile([128, 2 * SCH], BF16, name="e2m", tag="e2m")
                        nc.vector.tensor_mul(e2m[:], e2[:], masks2[jj][:])
                        e2 = e2m
                    e2s[p % 2] = e2
                if p > 0:
                    q = p - 1
                    e2 = e2s[q % 2]
                    for half in range(2):
                        tb = 2 * q + half
                        esl = e2[:, half * SCH:(half + 1) * SCH]
                        nc.tensor.matmul(
                            o_ps[:],
                            vN_c[tb // 4][:, (tb % 4) * 128:(tb % 4 + 1) * 128],
                            esl, start=(tb == 0), stop=(tb == ntb - 1),
                            skip_group_check=True)
                        nc.tensor.matmul(den_ps[:], ones_b[:], esl,
                                         start=(tb == 0), stop=(tb == ntb - 1),
                                         skip_group_check=True)
            dinv = atw.tile([128, SCH], F32, name="dinv", tag="dinv")
            nc.vector.reciprocal(dinv[:], den_ps[:])
            aT = atw.tile([128, SCH], BF16, name="aT", tag="aT")
            nc.vector.tensor_mul(aT[:], o_ps[:], dinv[:])
            # scatter into a2a_in: slab i = c//2, 4 shard writes of 128 cols
            i = c // 2
            j0 = 4 * (c % 2)
            for jj in range(4):
                j = j0 + jj
                nc.sync.dma_start(
                    a2a_in[i][(256 * j + 128 * h):(256 * j + 128 * (h + 1)), :],
                    aT[:, jj * 128:(jj + 1) * 128])

        if c % 2 == 1:
            i = c // 2
            nc.gpsimd.collective_compute(
                "AllToAll", ALU.bypass, replica_groups=rg,
                ins=[a2a_in[i][:].opt()], outs=[a2a_out[i][:].opt()])

    pb_st.close()
    ab.close()

    # ============ Phase C: o_proj + residual + ln2 (own 512 cols) ============
    cd = ExitStack()
    ps_op = cd.enter_context(tc.tile_pool(name="ps_op", bufs=2, space="PSUM"))
    ps_s2 = cd.enter_context(tc.tile_pool(name="ps_s2", bufs=1, space="PSUM"))
    ps_g = cd.enter_context(tc.tile_pool(name="ps_g", bufs=1, space="PSUM"))
    ps_u = cd.enter_context(tc.tile_pool(name="ps_u", bufs=1, space="PSUM"))
    ps_dn = cd.enter_context(tc.tile_pool(name="ps_dn", bufs=2, space="PSUM"))
    ps_t2 = cd.enter_context(tc.tile_pool(name="ps_t2", bufs=1, space="PSUM"))

    h2p = cd.enter_context(tc.tile_pool(name="h2p", bufs=1))
    xnp = cd.enter_context(tc.tile_pool(name="xnp", bufs=1))
    dw = cd.enter_context(tc.tile_pool(name="dw", bufs=2))

    h2s = []
    xns = []
    with tc.tile_pool(name="arp", bufs=1) as arp, \
         tc.tile_pool(name="rp", bufs=1) as rp, \
         tc.tile_pool(name="wop", bufs=2) as wop:
        rTs = []
        for hb in range(HB):
            rT = rp.tile([128, OWN], F32, name="rT", tag=f"rT{hb}")
            nc.sync.dma_start(rT[:], resT[hb * 128:(hb + 1) * 128, :])
            rTs.append(rT)
        attn_rhs = []
        for fb in range(HB):
            ar = arp.tile([128, OWN], BF16, name="ar", tag=f"ar{fb}")
            for i in range(NSLAB):
                nc.sync.dma_start(ar[:, i * SLAB:(i + 1) * SLAB],
                                  a2a_out[i][fb * 128:(fb + 1) * 128, :])
            attn_rhs.append(ar)

        ss2_ps = ps_s2.tile([128, OWN], F32, name="ss2_ps", tag="s2")
        for n in range(HB):
            won = wop.tile([128, HID], BF16, name="won", tag="won")
            nc.sync.dma_start(won[:], wo_t[n * 128:(n + 1) * 128, :])
            o_acc = ps_op.tile([128, OWN], F32, name="o_acc", tag="op")
            for k in range(HB):
                nc.tensor.matmul(o_acc[:], won[:, k * 128:(k + 1) * 128],
                                 attn_rhs[k][:],
                                 start=(k == 0), stop=(k == HB - 1),
                                 skip_group_check=True)
            h2 = h2p.tile([128, OWN], F32, name="h2", tag=f"h2{n}")
            nc.vector.tensor_add(h2[:], o_acc[:], rTs[n][:])
            sq2 = dw.tile([128, OWN], BF16, name="sq2", tag="sq2")
            nc.gpsimd.tensor_mul(sq2[:], h2[:], h2[:])
            nc.tensor.matmul(ss2_ps[:], ones_b[:], sq2[:],
                             start=(n == 0), stop=(n == HB - 1),
                             skip_group_check=True)
            h2s.append(h2)

        std2 = dw.tile([128, OWN], F32, name="std2", tag="std2")
        nc.scalar.activation(std2[:], ss2_ps[:], AF.Sqrt, scale=1.0 / HID,
                             bias=epsc[:, :])
        rinv2_bc = dw.tile([128, OWN], F32, name="rinv2_bc", tag="r2bc")
        nc.vector.reciprocal(rinv2_bc[:], std2[:])
        for n in range(HB):
            xn = xnp.tile([128, OWN], BF16, name="xn", tag=f"xn{n}")
            nc.vector.tensor_mul(xn[:], h2s[n][:], rinv2_bc[:])
            xns.append(xn)

    # ============ Phase D: gate/up + silu*up + down + residual + out ========
    with tc.tile_pool(name="wgup", bufs=3) as wgup, \
         tc.tile_pool(name="wdnp", bufs=2) as wdnp, \
         tc.tile_pool(name="hTp", bufs=1) as hTp:
        hTs = []
        for m in range(MB):
            wgum = wgup.tile([128, 2 * HID], BF16, name="wgum", tag="wgum")
            nc.sync.dma_start(wgum[:], wgu_t[m * 128:(m + 1) * 128, :])
            g_ps = ps_g.tile([128, OWN], F32, name="g_ps", tag="g")
            for hb in range(HB):
                nc.tensor.matmul(g_ps[:], wgum[:, hb * 128:(hb + 1) * 128],
                                 xns[hb][:],
                                 start=(hb == 0), stop=(hb == HB - 1),
                                 skip_group_check=True)
            u_ps = ps_u.tile([128, OWN], F32, name="u_ps", tag="u")
            for hb in range(HB):
                nc.tensor.matmul(u_ps[:], wgum[:, HID + hb * 128:HID + (hb + 1) * 128],
                                 xns[hb][:],
                                 start=(hb == 0), stop=(hb == HB - 1),
                                 skip_group_check=True)
            sg = dw.tile([128, OWN], F32, name="sg", tag="sg")
            nc.scalar.activation(sg[:], g_ps[:], AF.Silu)
            hT = hTp.tile([128, OWN], BF16, name="hT", tag=f"hT{m}")
            nc.vector.tensor_mul(hT[:], sg[:], u_ps[:])
            hTs.append(hT)

        for n in range(HB):
            wdnn = wdnp.tile([128, INTER], BF16, name="wdnn", tag="wdnn")
            nc.sync.dma_start(wdnn[:], wdn_t[n * 128:(n + 1) * 128, :])
            dn_ps = ps_dn.tile([128, OWN], F32, name="dn_ps", tag="dn")
            for k in range(MB):
                nc.tensor.matmul(dn_ps[:], wdnn[:, k * 128:(k + 1) * 128],
                                 hTs[k][:],
                                 start=(k == 0), stop=(k == MB - 1),
                                 skip_group_check=True)
            y = dw.tile([128, OWN], F32R, name="y", tag="y")
            nc.vector.tensor_add(y[:], dn_ps[:], h2s[n][:])
            for qb in range(OWN // 128):
                tp = ps_t2.tile([128, 128], F32R, name="tp2", tag="t2")
                nc.tensor.transpose(tp[:], y[:, qb * 128:(qb + 1) * 128], identr[:])
                oc = dw.tile([128, 128], F32, name="oc", tag="oc")
                nc.vector.tensor_copy(oc[:], tp.bitcast(F32)[:])
                nc.sync.dma_start(
                    out[qb * 128:(qb + 1) * 128, n * 128:(n + 1) * 128], oc[:])

    cd.close()
    es.close()


# ---------------- host side ----------------

_CACHE = {}


def _get_runner():
    if "runner" in _CACHE:
        return _CACHE["runner"]
    import jax
    from jax.sharding import Mesh, PartitionSpec
    from jax.experimental.shard_map import shard_map
    from concourse import bass2jax

    nc = _build()
    bass2jax.install_neuronx_cc_hook()

    in_names = []
    out_names = []
    out_avals = []
    zero_shapes = []
    for alloc in nc.m.functions[0].allocations:
        if not isinstance(alloc, mybir.MemoryLocationSet):
            continue
        name = alloc.memorylocations[0].name
        if alloc.kind == "ExternalInput":
            if nc.partition_id_tensor is None or name != nc.partition_id_tensor.name:
                in_names.append(name)
        elif alloc.kind == "ExternalOutput":
            out_names.append(name)
            shape = tuple(alloc.tensor_shape)
            dtype = mybir.dt.np(alloc.dtype)
            out_avals.append(jax.core.ShapedArray(shape, dtype))
            zero_shapes.append((shape, dtype))
    n_params = len(in_names)
    full_in_names = list(in_names) + list(out_names)
    if nc.partition_id_tensor is not None:
        full_in_names.append(nc.partition_id_tensor.name)
    donate = tuple(range(n_params, n_params + len(out_names)))

    def _body(*args):
        operands = list(args)
        if nc.partition_id_tensor is not None:
            operands.append(bass2jax.partition_id_tensor())
        outs = bass2jax._bass_exec_p.bind(
            *operands,
            out_avals=tuple(out_avals),
            in_names=tuple(full_in_names),
            out_names=tuple(out_names),
            lowering_input_output_aliases=(),
            sim_require_finite=True,
            sim_require_nnan=True,
            nc=nc,
        )
        return tuple(outs)

    devices = jax.devices()[:NCORES]
    mesh = Mesh(np.asarray(devices), ("core",))
    in_specs = (PartitionSpec("core"),) * (n_params + len(out_names))
    out_specs = (PartitionSpec("core"),) * len(out_names)
    sharded = jax.jit(
        shard_map(_body, mesh=mesh, in_specs=in_specs, out_specs=out_specs,
                  check_rep=False),
        donate_argnums=donate, keep_unused=True,
    )
    runner = dict(fn=sharded, in_names=in_names, out_names=out_names,
                  zero_shapes=zero_shapes, out_avals=out_avals)
    _CACHE["runner"] = runner
    return runner


def _prep_inputs(positions, hidden_states, ln1_w, ln2_w, w_qkv, w_o, w_gate_up, w_down):
    """Build per-core input dicts (list of NCORES dicts, numpy)."""
    import ml_dtypes
    BF = ml_dtypes.bfloat16
    hs = np.asarray(hidden_states, dtype=np.float32)
    pos = np.asarray(positions, dtype=np.float64)
    ln1 = np.asarray(ln1_w, dtype=np.float32)
    ln2 = np.asarray(ln2_w, dtype=np.float32)
    wq = np.asarray(w_qkv, dtype=np.float32)
    wo = np.asarray(w_o, dtype=np.float32)
    wgu = np.asarray(w_gate_up, dtype=np.float32)
    wdn = np.asarray(w_down, dtype=np.float32)

    hidT = np.ascontiguousarray(hs.T)                      # [HID, SEQ] f32
    hidT_bf = hidT.astype(BF)
    inv_freq = 1.0 / (THETA ** (np.arange(0, HD, 2, dtype=np.float64) / HD))
    freqs = pos[:, None] * inv_freq[None, :]               # [SEQ, 64]
    cos_h = np.cos(freqs).T.astype(np.float32)             # [64, SEQ]
    sin_h = np.sin(freqs).T.astype(np.float32)
    cos_t = np.ascontiguousarray(np.concatenate([cos_h, cos_h], axis=0))
    sin_t = np.ascontiguousarray(np.concatenate([-sin_h, sin_h], axis=0))

    wq_eff = wq * ln1[:, None]
    wgu_eff = wgu * ln2[:, None]
    scale = HD ** -0.5
    q_size = NH * HD
    kv_size = NKV * HD

    # shared pre-tiled weights (lhsT block layouts)
    wo_t = np.ascontiguousarray(
        wo.reshape(HB, 128, HB, 128).transpose(2, 1, 0, 3).reshape(HID, HID)
    ).astype(BF)
    G = wgu_eff[:, :INTER].reshape(HB, 128, MB, 128).transpose(2, 1, 0, 3)
    U = wgu_eff[:, INTER:].reshape(HB, 128, MB, 128).transpose(2, 1, 0, 3)
    wgu_t = np.ascontiguousarray(
        np.stack([G, U], axis=2).reshape(INTER, 2 * HID)).astype(BF)
    wdn_t = np.ascontiguousarray(
        wdn.reshape(MB, 128, HB, 128).transpose(2, 1, 0, 3).reshape(HID, INTER)
    ).astype(BF)

    per_core = []
    for c in range(NCORES):
        kvh = c // 2
        q_cols = wq_eff[:, QH * c * HD:QH * (c + 1) * HD] * scale
        k_cols = wq_eff[:, q_size + kvh * HD:q_size + (kvh + 1) * HD]
        v_cols = wq_eff[:, q_size + kv_size + kvh * HD:q_size + kv_size + (kvh + 1) * HD]
        W = np.concatenate([q_cols, k_cols, v_cols], axis=1)   # [HID, 512]
        w_qkv_s = np.ascontiguousarray(
            W.reshape(HB, 128, QH + 2, 128).transpose(2, 1, 0, 3)
            .reshape((QH + 2) * 128, HID)).astype(BF)
        own = np.concatenate(
            [hidT[:, (SEQ // NSLAB) * i + SLAB * c:(SEQ // NSLAB) * i + SLAB * (c + 1)]
             for i in range(NSLAB)], axis=1)
        per_core.append({
            "hidT_bf": hidT_bf, "resT": np.ascontiguousarray(own),
            "cos_t": cos_t, "sin_t": sin_t,
            "w_qkv_s": w_qkv_s, "wo_t": wo_t, "wgu_t": wgu_t, "wdn_t": wdn_t,
        })
    return per_core


def kernel(positions, hidden_states, ln1_w, ln2_w, w_qkv, w_o, w_gate_up, w_down):
    runner = _get_runner()
    per_core = _prep_inputs(positions, hidden_states, ln1_w, ln2_w,
                            w_qkv, w_o, w_gate_up, w_down)
    concat_in = [
        np.concatenate([np.asarray(per_core[c][name]) for c in range(NCORES)], axis=0)
        for name in runner["in_names"]
    ]
    concat_zeros = [
        np.zeros((NCORES * s[0],) + tuple(s[1:]), d)
        for (s, d) in runner["zero_shapes"]
    ]
    outs = runner["fn"](*concat_in, *concat_zeros)
    out = np.asarray(outs[0]).reshape(NCORES, OWN, HID)
    full = np.empty((SEQ, HID), dtype=np.float32)
    for c in range(NCORES):
        for i in range(NSLAB):
            full[(SEQ // NSLAB) * i + SLAB * c:(SEQ // NSLAB) * i + SLAB * (c + 1)] = \
                out[c][SLAB * i:SLAB * (i + 1)]
    return full


if __name__ == "__main__":
    print("building...")
    _get_runner()
    print("built ok")
